# revision 36
# baseline (speedup 1.0000x reference)
"""MoE (8 experts, top-2) TRN2 kernel — expert-parallel with routed dispatch.

Host computes the (tiny, 0.2%-of-FLOPs) gating softmax + top-2 routing and
shards tokens by expert. Load-balancing: each core holds TWO expert weight
sets — a primary (A) serving n_a groups of 256 tokens and an overflow (B)
serving one final 128-token group — so the per-core budget is 33 tiles of 128
(the exact balanced total of 259 tiles, +5 pad) instead of the 36 tiles the
heaviest expert alone would force. The first matmul runs two 4-pass chains
into one PSUM bank and applies ONE gelu per pair (when b1 == 0), keeping the
scalar engine comfortably ahead of the PE. Tokens arrive pre-gathered and
transposed in the matmul-friendly [128, 4, C] bf16 layout with per-token
combine weights; each core writes compact comb-weighted output rows and the
host scatter-adds the 8 compact outputs into the full [T, H] result.
"""

import sys
import types

sys.path.insert(0, "/opt/trn_rl_repo")

import numpy as np
import ml_dtypes

try:
    import antenv.axon_hooks  # noqa: F401
except ImportError:
    try:
        import antenv
        import trn_agent_boot.trn_boot as _tb

        _hook = _tb._ntff_profile_via_ctypes("/opt/axon/libaxon_pjrt.so")
        _m = types.ModuleType("antenv.axon_hooks")
        _m.get_axon_ntff_profile_hook = lambda: _hook
        _m.set_axon_ntff_profile_hook = lambda h: None
        sys.modules["antenv.axon_hooks"] = _m
        antenv.axon_hooks = _m
    except Exception:
        pass

import concourse.bacc as bacc
import concourse.mybir as mybir
from concourse import bass, bass_utils
from concourse.tile import TileContext

E = 8
H = 512
F = 2048
T = 8 * 2048
AW = 256  # tokens per A-group
BW = 128  # tokens in the single B-group
BF16 = mybir.dt.bfloat16
F32 = mybir.dt.float32

_CACHE = {}
LAST_RESULT = None


def _build(n_a, b1_zero, b2_zero):
    """FFN over n_a*AW + BW routed tokens; last group uses weight set B."""
    C = n_a * AW + BW
    nc = bacc.Bacc(debug=False)

    xg = nc.declare_dram_parameter("xg", [128, 4, C], BF16, isOutput=False)
    w1a = nc.declare_dram_parameter("w1a", [128, 4, F], BF16, isOutput=False)
    w2a = nc.declare_dram_parameter("w2a", [128, F // 128, H], BF16, isOutput=False)
    w1b = nc.declare_dram_parameter("w1b", [128, 4, F], BF16, isOutput=False)
    w2b = nc.declare_dram_parameter("w2b", [128, F // 128, H], BF16, isOutput=False)
    b1t = nc.declare_dram_parameter("b1t", [2, 128, F // 128], F32, isOutput=False)
    b2r = nc.declare_dram_parameter("b2r", [2, 128, H], F32, isOutput=False)
    comb = nc.declare_dram_parameter("comb", [128, C // 128], F32, isOutput=False)
    yout = nc.declare_dram_parameter("yout", [C, H], BF16, isOutput=True)

    # groups: (token_base, width, wset) with wset 0=A 1=B
    groups = [(i * AW, AW, 0) for i in range(n_a)] + [(n_a * AW, BW, 1)]
    n_groups = len(groups)

    with TileContext(nc) as tc:
        with (
            tc.tile_pool(name="const", bufs=1) as constp,
            tc.tile_pool(name="work", bufs=4) as work,
            tc.tile_pool(name="hpool", bufs=3) as hpool,
            tc.tile_pool(name="psA", bufs=4, space="PSUM") as psA,
            tc.tile_pool(name="psB", bufs=4, space="PSUM") as psB,
        ):
            # warmup: dep-free matmuls keep the PE clock ramping while the
            # first weight blocks stream in (preamble is HBM-bound)
            warm = constp.tile([128, 256], BF16)
            nc.vector.memset(warm[:], 0)
            wp = psB.tile([128, 256], F32, tag="mmB")
            for _ in range(10):
                nc.tensor.matmul(wp[:], warm[:, 0:128], warm[:], start=True, stop=True)

            # critical path: stream w1a in ft-blocks across ALL THREE dma
            # queues (sync + scalar + gpsimd) — the preamble is chip-HBM
            # bound with ~110GB/s per queue while all 8 cores load weights.
            NB = 16  # w1 ft-blocks
            FB = F // NB
            w1a_sb = [
                constp.tile([128, 4, FB], BF16, name=f"w1a_{b}") for b in range(NB)
            ]
            xg_first = work.tile([128, 4, AW], BF16, tag="xg")
            nc.scalar.dma_start(out=xg_first[:], in_=xg[:, :, 0:AW])
            for b in range(NB):
                eng = nc.scalar if b % 2 else nc.sync
                eng.dma_start(out=w1a_sb[b][:], in_=w1a[:, :, b * FB : (b + 1) * FB])

            # small consts behind the gpsimd w1a blocks
            b1_sb = [
                constp.tile([128, F // 128], F32, name=f"b1_{s}") for s in range(2)
            ]
            b2_sb = [constp.tile([128, H], F32, name=f"b2_{s}") for s in range(2)]
            if not b1_zero:
                for s in range(2):
                    nc.gpsimd.dma_start(out=b1_sb[s][:], in_=b1t[s])
            if not b2_zero:
                for s in range(2):
                    nc.gpsimd.dma_start(out=b2_sb[s][:], in_=b2r[s])
            comb_sb = constp.tile([128, C // 128], F32)
            nc.gpsimd.dma_start(out=comb_sb[:], in_=comb[:])
            # prefetch the next two x groups on the scalar queue so they don't
            # sit behind the 2MB w2a transfer on the sync ring
            xg_pre = {}
            for g in (1, 2):
                t = work.tile([128, 4, AW], BF16, tag="xg", name=f"xg_pre{g}")
                nc.scalar.dma_start(out=t[:], in_=xg[:, :, g * AW : (g + 1) * AW])
                xg_pre[g] = t
            w2a_sb = constp.tile([128, F // 128, H], BF16)
            for q in range(4):
                nc.sync.dma_start(
                    out=w2a_sb[:, 4 * q : 4 * (q + 1), :],
                    in_=w2a[:, 4 * q : 4 * (q + 1), :],
                )
            # B weight set tiles; transfers are emitted later, spread across
            # early groups on the scalar queue (slack until the last group).
            w1b_sb = [
                constp.tile([128, 4, FB], BF16, name=f"w1b_{b}") for b in range(NB)
            ]
            w2b_sb = constp.tile([128, F // 128, H], BF16)

            def emit_bset_chunk(k):
                # spread the B-set transfers over calls k = 0..5
                if k < 4:
                    for b in range(NB // 4 * k, NB // 4 * (k + 1)):
                        nc.scalar.dma_start(
                            out=w1b_sb[b][:], in_=w1b[:, :, b * FB : (b + 1) * FB]
                        )
                else:
                    for q in ((0, 1) if k == 4 else (2, 3)):
                        nc.scalar.dma_start(
                            out=w2b_sb[:, 4 * q : 4 * (q + 1), :],
                            in_=w2b[:, 4 * q : 4 * (q + 1), :],
                        )

            FPB = FB // 128  # ft tiles per w1 block

            def emit_mm1(W, w1s, bsel, xg_sb):
                hb = hpool.tile([128, F // 128, AW], BF16, tag="hb")
                for p in range(8):
                    hp = psA.tile([128, 2, AW], F32, tag="mmA")
                    for half in range(2):
                        ft = 2 * p + half
                        w1blk = w1s[ft // FPB]
                        fo = (ft % FPB) * 128
                        for hc in range(4):
                            nc.tensor.matmul(
                                hp[:, half, 0:W],
                                w1blk[:, hc, fo : fo + 128],
                                xg_sb[:, hc, 0:W],
                                start=(hc == 0),
                                stop=(hc == 3),
                            )
                    if b1_zero:
                        nc.scalar.activation(
                            hb[:, 2 * p : 2 * p + 2, 0:W],
                            hp[:, :, 0:W],
                            mybir.ActivationFunctionType.Gelu_apprx_tanh,
                            bias=0.0,
                            scale=1.0,
                        )
                    else:
                        for half in range(2):
                            nc.scalar.activation(
                                hb[:, 2 * p + half, 0:W],
                                hp[:, half, 0:W],
                                mybir.ActivationFunctionType.Gelu_apprx_tanh,
                                bias=b1_sb[bsel][:, 2 * p + half : 2 * p + half + 1],
                                scale=1.0,
                            )
                return hb

            def emit_mm2(base, W, hb, w2s, bsel):
                # second matmul emitted token-major: lhsT = h chunk (stationary),
                # moving = W2 rows -> output rows are tokens, no transposes
                for st in range(W // 128):
                    yp = psB.tile([128, H], F32, tag="mmB")
                    for fc in range(F // 128):
                        nc.tensor.matmul(
                            yp[:],
                            hb[:, fc, st * 128 : (st + 1) * 128],
                            w2s[:, fc, :],
                            start=(fc == 0),
                            stop=(fc == F // 128 - 1),
                        )
                    y_sb = work.tile([128, H], BF16, tag="ysb")
                    col = base // 128 + st
                    if b2_zero:
                        nc.vector.tensor_scalar_mul(
                            y_sb[:], yp[:], comb_sb[:, col : col + 1]
                        )
                    else:
                        nc.vector.tensor_tensor(
                            out=y_sb[:],
                            in0=yp[:],
                            in1=b2_sb[bsel][:],
                            op=mybir.AluOpType.add,
                        )
                        nc.vector.tensor_scalar_mul(
                            y_sb[:], y_sb[:], comb_sb[:, col : col + 1]
                        )
                    nc.sync.dma_start(
                        out=yout[base + st * 128 : base + (st + 1) * 128, :],
                        in_=y_sb[:],
                    )

            def xg_tile(g):
                if g == 0:
                    return xg_first
                if g in xg_pre:
                    return xg_pre[g]
                base, W, _ = groups[g]
                t = work.tile([128, 4, AW], BF16, tag="xg")
                nc.sync.dma_start(out=t[:, :, 0:W], in_=xg[:, :, base : base + W])
                return t

            def wsel(g):
                if groups[g][2]:
                    return w1b_sb, w2b_sb[:], 1
                return w1a_sb, w2a_sb[:], 0

            # software pipeline: mm1 runs one group ahead of mm2 so mm2(g)
            # never waits on gelu(g); PE stays fed.
            w1s, _, bsel = wsel(0)
            hbs = {0: emit_mm1(groups[0][1], w1s, bsel, xg_first)}
            for g in range(n_groups):
                if g + 1 < n_groups:
                    w1s, _, bsel = wsel(g + 1)
                    hbs[g + 1] = emit_mm1(groups[g + 1][1], w1s, bsel, xg_tile(g + 1))
                _, w2s, bsel = wsel(g)
                emit_mm2(groups[g][0], groups[g][1], hbs.pop(g), w2s, bsel)
                if 2 <= g <= 7:
                    emit_bset_chunk(g - 2)
    nc.compile()
    return nc


def _route(x, Wg, bg):
    """Host gating: returns flat tokens, per-expert index lists, combine wts."""
    xf = np.asarray(x, dtype=np.float32).reshape(T, H)
    logits = xf @ np.asarray(Wg, dtype=np.float32) + np.asarray(bg, dtype=np.float32)
    m = logits.max(-1, keepdims=True)
    p = np.exp(logits - m)
    p /= p.sum(-1, keepdims=True)
    order = np.argsort(-p, axis=-1)
    topi = order[:, :2]
    mask = np.zeros_like(p, dtype=bool)
    np.put_along_axis(mask, topi, True, axis=-1)
    comb = (p * mask).astype(np.float32)  # [T, E] raw softmax prob, top-2 gated
    idx_lists = [np.nonzero(mask[:, e])[0] for e in range(E)]
    return xf, idx_lists, comb


def _pack(idx_lists, n_a):
    """Assign (expert, token) pairs to per-core A (n_a*AW) / B (BW) regions.

    Greedy: expert c's first A_cap tokens on core c; overflow pieces split
    into <=BW chunks, first-fit into the 8 B slots.
    """
    a_cap = n_a * AW
    cores = []
    pieces = []
    for e in range(E):
        ix = idx_lists[e]
        cores.append({"pe": e, "pix": ix[:a_cap], "be": e, "bix": ix[:0]})
        if len(ix) > a_cap:
            pieces.append((e, ix[a_cap:]))
    pieces.sort(key=lambda p: -len(p[1]))
    free = list(range(E))
    for e, rem in pieces:
        while len(rem) > 0:
            assert free, "overflow does not fit; raise n_a"
            c = free.pop(0)
            cores[c]["be"] = e
            cores[c]["bix"] = rem[:BW]
            rem = rem[BW:]
    return cores


def kernel(x, Wg, bg, W1, b1, W2, b2):
    global LAST_RESULT
    xf, idx_lists, comb = _route(x, Wg, bg)
    total_tiles = sum(-(-len(ix) // 128) for ix in idx_lists)
    n_a = max(-(-(total_tiles - E * (BW // 128)) // (E * (AW // 128))), 2)

    def b_slots_needed(n):
        return sum(-(-max(0, len(ix) - n * AW) // BW) for ix in idx_lists)

    while b_slots_needed(n_a) > E:
        n_a += 1
    C = n_a * AW + BW

    b1 = np.asarray(b1, dtype=np.float32)
    b2 = np.asarray(b2, dtype=np.float32)
    b1_zero = not np.any(b1)
    b2_zero = not np.any(b2)
    key = ("nc", n_a, b1_zero, b2_zero)
    if key not in _CACHE:
        _CACHE[key] = _build(n_a, b1_zero, b2_zero)
    nc = _CACHE[key]

    W1 = np.asarray(W1, dtype=np.float32)
    W2 = np.asarray(W2, dtype=np.float32)
    w1p = {}
    w2p = {}
    for e in range(E):
        w1p[e] = np.ascontiguousarray(
            np.transpose(W1[e].reshape(4, 128, F), (1, 0, 2)).astype(ml_dtypes.bfloat16)
        )
        w2p[e] = np.ascontiguousarray(
            np.transpose(W2[e].reshape(F // 128, 128, H), (1, 0, 2)).astype(
                ml_dtypes.bfloat16
            )
        )

    cores = _pack(idx_lists, n_a)
    a_cap = n_a * AW
    in_maps = []
    for cdesc in cores:
        pe, be = cdesc["pe"], cdesc["be"]
        pix, bix = cdesc["pix"], cdesc["bix"]
        pad = np.zeros(C, dtype=np.int64)
        pad[: len(pix)] = pix
        pad[a_cap : a_cap + len(bix)] = bix
        xe = xf[pad]  # [C, H] fp32 (pad rows = token 0, comb 0)
        xgc = np.ascontiguousarray(
            np.transpose(xe.T.reshape(4, 128, C), (1, 0, 2)).astype(ml_dtypes.bfloat16)
        )
        cw = np.zeros(C, dtype=np.float32)
        cw[: len(pix)] = comb[pix, pe]
        cw[a_cap : a_cap + len(bix)] = comb[bix, be]
        in_maps.append(
            {
                "xg": xgc,
                "w1a": w1p[pe],
                "w2a": w2p[pe],
                "w1b": w1p[be],
                "w2b": w2p[be],
                "b1t": np.ascontiguousarray(
                    np.stack([b1[pe], b1[be]]).reshape(2, F // 128, 128).swapaxes(1, 2)
                ),
                "b2r": np.ascontiguousarray(
                    np.broadcast_to(
                        np.stack([b2[pe], b2[be]])[:, None, :], (2, 128, H)
                    ).copy()
                ),
                # comb[p, j] pairs with output row j*128+p of yout
                "comb": np.ascontiguousarray(cw.reshape(C // 128, 128).T),
            }
        )

    import os

    trace = bool(os.environ.get("BASS_TRACE"))
    res = bass_utils.run_bass_kernel_spmd(
        nc, in_maps, core_ids=list(range(E)), trace=trace
    )
    LAST_RESULT = res
    out = np.zeros((T, H), dtype=np.float32)
    for cdesc in cores:
        y = np.asarray(res.results[cores.index(cdesc)]["yout"], dtype=np.float32)
        pix, bix = cdesc["pix"], cdesc["bix"]
        out[pix] += y[: len(pix)]
        if len(bix):
            out[bix] += y[a_cap : a_cap + len(bix)]
    return out.reshape(8, 2048, H)


# revision 37
# speedup vs baseline: 1.0128x; 1.0128x over previous
"""MoE (8 experts, top-2) TRN2 kernel — expert-parallel with routed dispatch.

Host computes the (tiny, 0.2%-of-FLOPs) gating softmax + top-2 routing and
shards tokens by expert. Load-balancing: each core holds TWO expert weight
sets — a primary (A) serving n_a groups of 256 tokens and an overflow (B)
serving one final 128-token group — so the per-core budget is 33 tiles of 128
(the exact balanced total of 259 tiles, +5 pad) instead of the 36 tiles the
heaviest expert alone would force. The first matmul runs two 4-pass chains
into one PSUM bank and applies ONE gelu per pair (when b1 == 0), keeping the
scalar engine comfortably ahead of the PE. Tokens arrive pre-gathered and
transposed in the matmul-friendly [128, 4, C] bf16 layout with per-token
combine weights; each core writes compact comb-weighted output rows and the
host scatter-adds the 8 compact outputs into the full [T, H] result.
"""

import sys
import types

sys.path.insert(0, "/opt/trn_rl_repo")

import numpy as np
import ml_dtypes

try:
    import antenv.axon_hooks  # noqa: F401
except ImportError:
    try:
        import antenv
        import trn_agent_boot.trn_boot as _tb

        _hook = _tb._ntff_profile_via_ctypes("/opt/axon/libaxon_pjrt.so")
        _m = types.ModuleType("antenv.axon_hooks")
        _m.get_axon_ntff_profile_hook = lambda: _hook
        _m.set_axon_ntff_profile_hook = lambda h: None
        sys.modules["antenv.axon_hooks"] = _m
        antenv.axon_hooks = _m
    except Exception:
        pass

import concourse.bacc as bacc
import concourse.mybir as mybir
from concourse import bass, bass_utils
from concourse.tile import TileContext

E = 8
H = 512
F = 2048
T = 8 * 2048
AW = 256  # tokens per A-group
BW = 128  # tokens in the single B-group
BF16 = mybir.dt.bfloat16
F32 = mybir.dt.float32

_CACHE = {}
LAST_RESULT = None


def _build(n_a, b1_zero, b2_zero):
    """FFN over n_a*AW + BW routed tokens; last group uses weight set B."""
    C = n_a * AW + BW
    nc = bacc.Bacc(debug=False)

    xg = nc.declare_dram_parameter("xg", [128, 4, C], BF16, isOutput=False)
    w1a = nc.declare_dram_parameter("w1a", [128, 4, F], BF16, isOutput=False)
    w2a = nc.declare_dram_parameter("w2a", [128, F // 128, H], BF16, isOutput=False)
    w1b = nc.declare_dram_parameter("w1b", [128, 4, F], BF16, isOutput=False)
    w2b = nc.declare_dram_parameter("w2b", [128, F // 128, H], BF16, isOutput=False)
    b1t = nc.declare_dram_parameter("b1t", [2, 128, F // 128], F32, isOutput=False)
    b2r = nc.declare_dram_parameter("b2r", [2, 128, H], F32, isOutput=False)
    comb = nc.declare_dram_parameter("comb", [128, C // 128], F32, isOutput=False)
    yout = nc.declare_dram_parameter("yout", [C, H], BF16, isOutput=True)

    # groups: (token_base, width, wset) with wset 0=A 1=B
    groups = [(i * AW, AW, 0) for i in range(n_a)] + [(n_a * AW, BW, 1)]
    n_groups = len(groups)

    with TileContext(nc) as tc:
        with (
            tc.tile_pool(name="const", bufs=1) as constp,
            tc.tile_pool(name="work", bufs=4) as work,
            tc.tile_pool(name="hpool", bufs=3) as hpool,
            tc.tile_pool(name="psA", bufs=4, space="PSUM") as psA,
            tc.tile_pool(name="psB", bufs=4, space="PSUM") as psB,
        ):
            # warmup: dep-free matmuls keep the PE clock ramping while the
            # first weight blocks stream in (preamble is HBM-bound)
            warm = constp.tile([128, 256], BF16)
            nc.vector.memset(warm[:], 0)
            wp = psB.tile([128, 256], F32, tag="mmB")
            for _ in range(10):
                nc.tensor.matmul(wp[:], warm[:, 0:128], warm[:], start=True, stop=True)

            # critical path: stream w1a in ft-blocks across ALL THREE dma
            # queues (sync + scalar + gpsimd) — the preamble is chip-HBM
            # bound with ~110GB/s per queue while all 8 cores load weights.
            NB = 8  # w1 ft-blocks
            FB = F // NB
            w1a_sb = [
                constp.tile([128, 4, FB], BF16, name=f"w1a_{b}") for b in range(NB)
            ]
            xg_first = work.tile([128, 4, AW], BF16, tag="xg")
            nc.scalar.dma_start(out=xg_first[:], in_=xg[:, :, 0:AW])
            for b in range(NB):
                eng = nc.scalar if b % 2 else nc.sync
                eng.dma_start(out=w1a_sb[b][:], in_=w1a[:, :, b * FB : (b + 1) * FB])

            # small consts behind the gpsimd w1a blocks
            b1_sb = [
                constp.tile([128, F // 128], F32, name=f"b1_{s}") for s in range(2)
            ]
            b2_sb = [constp.tile([128, H], F32, name=f"b2_{s}") for s in range(2)]
            if not b1_zero:
                for s in range(2):
                    nc.gpsimd.dma_start(out=b1_sb[s][:], in_=b1t[s])
            if not b2_zero:
                for s in range(2):
                    nc.gpsimd.dma_start(out=b2_sb[s][:], in_=b2r[s])
            comb_sb = constp.tile([128, C // 128], F32)
            nc.gpsimd.dma_start(out=comb_sb[:], in_=comb[:])
            # prefetch the next two x groups on the scalar queue so they don't
            # sit behind the 2MB w2a transfer on the sync ring
            xg_pre = {}
            for g in (1, 2):
                t = work.tile([128, 4, AW], BF16, tag="xg", name=f"xg_pre{g}")
                nc.scalar.dma_start(out=t[:], in_=xg[:, :, g * AW : (g + 1) * AW])
                xg_pre[g] = t
            w2a_sb = constp.tile([128, F // 128, H], BF16)
            for q in range(4):
                nc.sync.dma_start(
                    out=w2a_sb[:, 4 * q : 4 * (q + 1), :],
                    in_=w2a[:, 4 * q : 4 * (q + 1), :],
                )
            # B weight set tiles; transfers are emitted later, spread across
            # early groups on the scalar queue (slack until the last group).
            w1b_sb = [
                constp.tile([128, 4, FB], BF16, name=f"w1b_{b}") for b in range(NB)
            ]
            w2b_sb = constp.tile([128, F // 128, H], BF16)

            def emit_bset_chunk(k):
                # spread the B-set transfers over calls k = 0..5
                if k < 4:
                    for b in range(NB // 4 * k, NB // 4 * (k + 1)):
                        nc.scalar.dma_start(
                            out=w1b_sb[b][:], in_=w1b[:, :, b * FB : (b + 1) * FB]
                        )
                else:
                    for q in ((0, 1) if k == 4 else (2, 3)):
                        nc.scalar.dma_start(
                            out=w2b_sb[:, 4 * q : 4 * (q + 1), :],
                            in_=w2b[:, 4 * q : 4 * (q + 1), :],
                        )

            FPB = FB // 128  # ft tiles per w1 block

            def emit_mm1(W, w1s, bsel, xg_sb):
                hb = hpool.tile([128, F // 128, AW], BF16, tag="hb")
                for p in range(8):
                    hp = psA.tile([128, 2, AW], F32, tag="mmA")
                    for half in range(2):
                        ft = 2 * p + half
                        w1blk = w1s[ft // FPB]
                        fo = (ft % FPB) * 128
                        for hc in range(4):
                            nc.tensor.matmul(
                                hp[:, half, 0:W],
                                w1blk[:, hc, fo : fo + 128],
                                xg_sb[:, hc, 0:W],
                                start=(hc == 0),
                                stop=(hc == 3),
                            )
                    if b1_zero:
                        nc.scalar.activation(
                            hb[:, 2 * p : 2 * p + 2, 0:W],
                            hp[:, :, 0:W],
                            mybir.ActivationFunctionType.Gelu_apprx_tanh,
                            bias=0.0,
                            scale=1.0,
                        )
                    else:
                        for half in range(2):
                            nc.scalar.activation(
                                hb[:, 2 * p + half, 0:W],
                                hp[:, half, 0:W],
                                mybir.ActivationFunctionType.Gelu_apprx_tanh,
                                bias=b1_sb[bsel][:, 2 * p + half : 2 * p + half + 1],
                                scale=1.0,
                            )
                return hb

            def emit_mm2(base, W, hb, w2s, bsel):
                # second matmul emitted token-major: lhsT = h chunk (stationary),
                # moving = W2 rows -> output rows are tokens, no transposes
                for st in range(W // 128):
                    yp = psB.tile([128, H], F32, tag="mmB")
                    for fc in range(F // 128):
                        nc.tensor.matmul(
                            yp[:],
                            hb[:, fc, st * 128 : (st + 1) * 128],
                            w2s[:, fc, :],
                            start=(fc == 0),
                            stop=(fc == F // 128 - 1),
                        )
                    y_sb = work.tile([128, H], BF16, tag="ysb")
                    col = base // 128 + st
                    if b2_zero:
                        nc.vector.tensor_scalar_mul(
                            y_sb[:], yp[:], comb_sb[:, col : col + 1]
                        )
                    else:
                        nc.vector.tensor_tensor(
                            out=y_sb[:],
                            in0=yp[:],
                            in1=b2_sb[bsel][:],
                            op=mybir.AluOpType.add,
                        )
                        nc.vector.tensor_scalar_mul(
                            y_sb[:], y_sb[:], comb_sb[:, col : col + 1]
                        )
                    nc.sync.dma_start(
                        out=yout[base + st * 128 : base + (st + 1) * 128, :],
                        in_=y_sb[:],
                    )

            def xg_tile(g):
                if g == 0:
                    return xg_first
                if g in xg_pre:
                    return xg_pre[g]
                base, W, _ = groups[g]
                t = work.tile([128, 4, AW], BF16, tag="xg")
                nc.sync.dma_start(out=t[:, :, 0:W], in_=xg[:, :, base : base + W])
                return t

            def wsel(g):
                if groups[g][2]:
                    return w1b_sb, w2b_sb[:], 1
                return w1a_sb, w2a_sb[:], 0

            # software pipeline: mm1 runs one group ahead of mm2 so mm2(g)
            # never waits on gelu(g); PE stays fed.
            w1s, _, bsel = wsel(0)
            hbs = {0: emit_mm1(groups[0][1], w1s, bsel, xg_first)}
            for g in range(n_groups):
                if g + 1 < n_groups:
                    w1s, _, bsel = wsel(g + 1)
                    hbs[g + 1] = emit_mm1(groups[g + 1][1], w1s, bsel, xg_tile(g + 1))
                _, w2s, bsel = wsel(g)
                emit_mm2(groups[g][0], groups[g][1], hbs.pop(g), w2s, bsel)
                if 2 <= g <= 7:
                    emit_bset_chunk(g - 2)
    nc.compile()
    return nc


def _route(x, Wg, bg):
    """Host gating: returns flat tokens, per-expert index lists, combine wts."""
    xf = np.asarray(x, dtype=np.float32).reshape(T, H)
    logits = xf @ np.asarray(Wg, dtype=np.float32) + np.asarray(bg, dtype=np.float32)
    m = logits.max(-1, keepdims=True)
    p = np.exp(logits - m)
    p /= p.sum(-1, keepdims=True)
    order = np.argsort(-p, axis=-1)
    topi = order[:, :2]
    mask = np.zeros_like(p, dtype=bool)
    np.put_along_axis(mask, topi, True, axis=-1)
    comb = (p * mask).astype(np.float32)  # [T, E] raw softmax prob, top-2 gated
    idx_lists = [np.nonzero(mask[:, e])[0] for e in range(E)]
    return xf, idx_lists, comb


def _pack(idx_lists, n_a):
    """Assign (expert, token) pairs to per-core A (n_a*AW) / B (BW) regions.

    Greedy: expert c's first A_cap tokens on core c; overflow pieces split
    into <=BW chunks, first-fit into the 8 B slots.
    """
    a_cap = n_a * AW
    cores = []
    pieces = []
    for e in range(E):
        ix = idx_lists[e]
        cores.append({"pe": e, "pix": ix[:a_cap], "be": e, "bix": ix[:0]})
        if len(ix) > a_cap:
            pieces.append((e, ix[a_cap:]))
    pieces.sort(key=lambda p: -len(p[1]))
    free = list(range(E))
    for e, rem in pieces:
        while len(rem) > 0:
            assert free, "overflow does not fit; raise n_a"
            c = free.pop(0)
            cores[c]["be"] = e
            cores[c]["bix"] = rem[:BW]
            rem = rem[BW:]
    return cores


def kernel(x, Wg, bg, W1, b1, W2, b2):
    global LAST_RESULT
    xf, idx_lists, comb = _route(x, Wg, bg)
    total_tiles = sum(-(-len(ix) // 128) for ix in idx_lists)
    n_a = max(-(-(total_tiles - E * (BW // 128)) // (E * (AW // 128))), 2)

    def b_slots_needed(n):
        return sum(-(-max(0, len(ix) - n * AW) // BW) for ix in idx_lists)

    while b_slots_needed(n_a) > E:
        n_a += 1
    C = n_a * AW + BW

    b1 = np.asarray(b1, dtype=np.float32)
    b2 = np.asarray(b2, dtype=np.float32)
    b1_zero = not np.any(b1)
    b2_zero = not np.any(b2)
    key = ("nc", n_a, b1_zero, b2_zero)
    if key not in _CACHE:
        _CACHE[key] = _build(n_a, b1_zero, b2_zero)
    nc = _CACHE[key]

    W1 = np.asarray(W1, dtype=np.float32)
    W2 = np.asarray(W2, dtype=np.float32)
    w1p = {}
    w2p = {}
    for e in range(E):
        w1p[e] = np.ascontiguousarray(
            np.transpose(W1[e].reshape(4, 128, F), (1, 0, 2)).astype(ml_dtypes.bfloat16)
        )
        w2p[e] = np.ascontiguousarray(
            np.transpose(W2[e].reshape(F // 128, 128, H), (1, 0, 2)).astype(
                ml_dtypes.bfloat16
            )
        )

    cores = _pack(idx_lists, n_a)
    a_cap = n_a * AW
    in_maps = []
    for cdesc in cores:
        pe, be = cdesc["pe"], cdesc["be"]
        pix, bix = cdesc["pix"], cdesc["bix"]
        pad = np.zeros(C, dtype=np.int64)
        pad[: len(pix)] = pix
        pad[a_cap : a_cap + len(bix)] = bix
        xe = xf[pad]  # [C, H] fp32 (pad rows = token 0, comb 0)
        xgc = np.ascontiguousarray(
            np.transpose(xe.T.reshape(4, 128, C), (1, 0, 2)).astype(ml_dtypes.bfloat16)
        )
        cw = np.zeros(C, dtype=np.float32)
        cw[: len(pix)] = comb[pix, pe]
        cw[a_cap : a_cap + len(bix)] = comb[bix, be]
        in_maps.append(
            {
                "xg": xgc,
                "w1a": w1p[pe],
                "w2a": w2p[pe],
                "w1b": w1p[be],
                "w2b": w2p[be],
                "b1t": np.ascontiguousarray(
                    np.stack([b1[pe], b1[be]]).reshape(2, F // 128, 128).swapaxes(1, 2)
                ),
                "b2r": np.ascontiguousarray(
                    np.broadcast_to(
                        np.stack([b2[pe], b2[be]])[:, None, :], (2, 128, H)
                    ).copy()
                ),
                # comb[p, j] pairs with output row j*128+p of yout
                "comb": np.ascontiguousarray(cw.reshape(C // 128, 128).T),
            }
        )

    import os

    trace = bool(os.environ.get("BASS_TRACE"))
    res = bass_utils.run_bass_kernel_spmd(
        nc, in_maps, core_ids=list(range(E)), trace=trace
    )
    LAST_RESULT = res
    out = np.zeros((T, H), dtype=np.float32)
    for cdesc in cores:
        y = np.asarray(res.results[cores.index(cdesc)]["yout"], dtype=np.float32)
        pix, bix = cdesc["pix"], cdesc["bix"]
        out[pix] += y[: len(pix)]
        if len(bix):
            out[bix] += y[a_cap : a_cap + len(bix)]
    return out.reshape(8, 2048, H)


# revision 41
# speedup vs baseline: 1.0128x; 1.0000x over previous
"""MoE (8 experts, top-2) TRN2 kernel — expert-parallel with routed dispatch.

Host computes the (tiny, 0.2%-of-FLOPs) gating softmax + top-2 routing and
shards tokens by expert. Load-balancing: each core holds TWO expert weight
sets — a primary (A) serving n_a groups of 256 tokens and an overflow (B)
serving one final 128-token group — so the per-core budget is 33 tiles of 128
(the exact balanced total of 259 tiles, +5 pad) instead of the 36 tiles the
heaviest expert alone would force. The first matmul runs two 4-pass chains
into one PSUM bank and applies ONE gelu per pair (when b1 == 0), keeping the
scalar engine comfortably ahead of the PE. Tokens arrive pre-gathered and
transposed in the matmul-friendly [128, 4, C] bf16 layout with per-token
combine weights; each core writes compact comb-weighted output rows and the
host scatter-adds the 8 compact outputs into the full [T, H] result.
"""

import sys
import types

sys.path.insert(0, "/opt/trn_rl_repo")

import numpy as np
import ml_dtypes

try:
    import antenv.axon_hooks  # noqa: F401
except ImportError:
    try:
        import antenv
        import trn_agent_boot.trn_boot as _tb

        _hook = _tb._ntff_profile_via_ctypes("/opt/axon/libaxon_pjrt.so")
        _m = types.ModuleType("antenv.axon_hooks")
        _m.get_axon_ntff_profile_hook = lambda: _hook
        _m.set_axon_ntff_profile_hook = lambda h: None
        sys.modules["antenv.axon_hooks"] = _m
        antenv.axon_hooks = _m
    except Exception:
        pass

import concourse.bacc as bacc
import concourse.mybir as mybir
from concourse import bass, bass_utils
from concourse.tile import TileContext

E = 8
H = 512
F = 2048
T = 8 * 2048
AW = 256  # tokens per A-group
BW = 128  # tokens in the single B-group
BF16 = mybir.dt.bfloat16
F32 = mybir.dt.float32

_CACHE = {}
LAST_RESULT = None


def _build(n_a, b1_zero, b2_zero):
    """FFN over n_a*AW + BW routed tokens; last group uses weight set B."""
    C = n_a * AW + BW
    nc = bacc.Bacc(debug=False)

    xg = nc.declare_dram_parameter("xg", [128, 4, C], BF16, isOutput=False)
    w1a = nc.declare_dram_parameter("w1a", [128, 4, F], BF16, isOutput=False)
    w2a = nc.declare_dram_parameter("w2a", [128, F // 128, H], BF16, isOutput=False)
    w1b = nc.declare_dram_parameter("w1b", [128, 4, F], BF16, isOutput=False)
    w2b = nc.declare_dram_parameter("w2b", [128, F // 128, H], BF16, isOutput=False)
    b1t = nc.declare_dram_parameter("b1t", [2, 128, F // 128], F32, isOutput=False)
    b2r = nc.declare_dram_parameter("b2r", [2, 128, H], F32, isOutput=False)
    comb = nc.declare_dram_parameter("comb", [128, C // 128], F32, isOutput=False)
    yout = nc.declare_dram_parameter("yout", [C, H], F32, isOutput=True)

    # groups: (token_base, width, wset) with wset 0=A 1=B
    groups = [(i * AW, AW, 0) for i in range(n_a)] + [(n_a * AW, BW, 1)]
    n_groups = len(groups)

    with TileContext(nc) as tc:
        with (
            tc.tile_pool(name="const", bufs=1) as constp,
            tc.tile_pool(name="work", bufs=4) as work,
            tc.tile_pool(name="hpool", bufs=3) as hpool,
            tc.tile_pool(name="psA", bufs=4, space="PSUM") as psA,
            tc.tile_pool(name="psB", bufs=4, space="PSUM") as psB,
        ):
            # warmup: dep-free matmuls keep the PE clock ramping while the
            # first weight blocks stream in (preamble is HBM-bound)
            warm = constp.tile([128, 256], BF16)
            nc.vector.memset(warm[:], 0)
            wp = psB.tile([128, 256], F32, tag="mmB")
            for _ in range(10):
                nc.tensor.matmul(wp[:], warm[:, 0:128], warm[:], start=True, stop=True)

            # critical path: stream w1a in ft-blocks across ALL THREE dma
            # queues (sync + scalar + gpsimd) — the preamble is chip-HBM
            # bound with ~110GB/s per queue while all 8 cores load weights.
            NB = 8  # w1 ft-blocks
            FB = F // NB
            w1a_sb = [
                constp.tile([128, 4, FB], BF16, name=f"w1a_{b}") for b in range(NB)
            ]
            xg_first = work.tile([128, 4, AW], BF16, tag="xg")
            nc.scalar.dma_start(out=xg_first[:], in_=xg[:, :, 0:AW])
            for b in range(NB):
                eng = nc.scalar if b % 2 else nc.sync
                eng.dma_start(out=w1a_sb[b][:], in_=w1a[:, :, b * FB : (b + 1) * FB])

            # small consts behind the gpsimd w1a blocks
            b1_sb = [
                constp.tile([128, F // 128], F32, name=f"b1_{s}") for s in range(2)
            ]
            b2_sb = [constp.tile([128, H], F32, name=f"b2_{s}") for s in range(2)]
            if not b1_zero:
                for s in range(2):
                    nc.gpsimd.dma_start(out=b1_sb[s][:], in_=b1t[s])
            if not b2_zero:
                for s in range(2):
                    nc.gpsimd.dma_start(out=b2_sb[s][:], in_=b2r[s])
            comb_sb = constp.tile([128, C // 128], F32)
            nc.gpsimd.dma_start(out=comb_sb[:], in_=comb[:])
            # prefetch the next two x groups on the scalar queue so they don't
            # sit behind the 2MB w2a transfer on the sync ring
            xg_pre = {}
            for g in (1, 2):
                t = work.tile([128, 4, AW], BF16, tag="xg", name=f"xg_pre{g}")
                nc.scalar.dma_start(out=t[:], in_=xg[:, :, g * AW : (g + 1) * AW])
                xg_pre[g] = t
            w2a_sb = constp.tile([128, F // 128, H], BF16)
            for q in range(4):
                nc.sync.dma_start(
                    out=w2a_sb[:, 4 * q : 4 * (q + 1), :],
                    in_=w2a[:, 4 * q : 4 * (q + 1), :],
                )
            # B weight set tiles; transfers are emitted later, spread across
            # early groups on the scalar queue (slack until the last group).
            w1b_sb = [
                constp.tile([128, 4, FB], BF16, name=f"w1b_{b}") for b in range(NB)
            ]
            w2b_sb = constp.tile([128, F // 128, H], BF16)

            def emit_bset_chunk(k):
                # spread the B-set transfers over calls k = 0..5
                if k < 4:
                    for b in range(NB // 4 * k, NB // 4 * (k + 1)):
                        nc.scalar.dma_start(
                            out=w1b_sb[b][:], in_=w1b[:, :, b * FB : (b + 1) * FB]
                        )
                else:
                    for q in ((0, 1) if k == 4 else (2, 3)):
                        nc.scalar.dma_start(
                            out=w2b_sb[:, 4 * q : 4 * (q + 1), :],
                            in_=w2b[:, 4 * q : 4 * (q + 1), :],
                        )

            FPB = FB // 128  # ft tiles per w1 block

            def emit_mm1(W, w1s, bsel, xg_sb):
                hb = hpool.tile([128, F // 128, AW], BF16, tag="hb")
                for p in range(8):
                    hp = psA.tile([128, 2, AW], F32, tag="mmA")
                    for half in range(2):
                        ft = 2 * p + half
                        w1blk = w1s[ft // FPB]
                        fo = (ft % FPB) * 128
                        for hc in range(4):
                            nc.tensor.matmul(
                                hp[:, half, 0:W],
                                w1blk[:, hc, fo : fo + 128],
                                xg_sb[:, hc, 0:W],
                                start=(hc == 0),
                                stop=(hc == 3),
                            )
                    if b1_zero:
                        nc.scalar.activation(
                            hb[:, 2 * p : 2 * p + 2, 0:W],
                            hp[:, :, 0:W],
                            mybir.ActivationFunctionType.Gelu_apprx_tanh,
                            bias=0.0,
                            scale=1.0,
                        )
                    else:
                        for half in range(2):
                            nc.scalar.activation(
                                hb[:, 2 * p + half, 0:W],
                                hp[:, half, 0:W],
                                mybir.ActivationFunctionType.Gelu_apprx_tanh,
                                bias=b1_sb[bsel][:, 2 * p + half : 2 * p + half + 1],
                                scale=1.0,
                            )
                return hb

            def emit_mm2(base, W, hb, w2s, bsel):
                # second matmul emitted token-major: lhsT = h chunk (stationary),
                # moving = W2 rows -> output rows are tokens, no transposes
                for st in range(W // 128):
                    yp = psB.tile([128, H], F32, tag="mmB")
                    for fc in range(F // 128):
                        nc.tensor.matmul(
                            yp[:],
                            hb[:, fc, st * 128 : (st + 1) * 128],
                            w2s[:, fc, :],
                            start=(fc == 0),
                            stop=(fc == F // 128 - 1),
                        )
                    y_sb = work.tile([128, H], F32, tag="ysb")
                    col = base // 128 + st
                    if b2_zero:
                        nc.vector.tensor_scalar_mul(
                            y_sb[:], yp[:], comb_sb[:, col : col + 1]
                        )
                    else:
                        nc.vector.tensor_tensor(
                            out=y_sb[:],
                            in0=yp[:],
                            in1=b2_sb[bsel][:],
                            op=mybir.AluOpType.add,
                        )
                        nc.vector.tensor_scalar_mul(
                            y_sb[:], y_sb[:], comb_sb[:, col : col + 1]
                        )
                    nc.sync.dma_start(
                        out=yout[base + st * 128 : base + (st + 1) * 128, :],
                        in_=y_sb[:],
                    )

            def xg_tile(g):
                if g == 0:
                    return xg_first
                if g in xg_pre:
                    return xg_pre[g]
                base, W, _ = groups[g]
                t = work.tile([128, 4, AW], BF16, tag="xg")
                nc.sync.dma_start(out=t[:, :, 0:W], in_=xg[:, :, base : base + W])
                return t

            def wsel(g):
                if groups[g][2]:
                    return w1b_sb, w2b_sb[:], 1
                return w1a_sb, w2a_sb[:], 0

            # software pipeline: mm1 runs one group ahead of mm2 so mm2(g)
            # never waits on gelu(g); PE stays fed.
            # B-set transfer schedule: chunks 0..5 spread over early groups,
            # always finishing before the final (B) group needs them
            bset_at = {}
            for k in range(6):
                bset_at.setdefault(min(2 + k, n_groups - 2), []).append(k)

            w1s, _, bsel = wsel(0)
            hbs = {0: emit_mm1(groups[0][1], w1s, bsel, xg_first)}
            for g in range(n_groups):
                if g + 1 < n_groups:
                    w1s, _, bsel = wsel(g + 1)
                    hbs[g + 1] = emit_mm1(groups[g + 1][1], w1s, bsel, xg_tile(g + 1))
                _, w2s, bsel = wsel(g)
                emit_mm2(groups[g][0], groups[g][1], hbs.pop(g), w2s, bsel)
                for k in bset_at.get(g, []):
                    emit_bset_chunk(k)
    nc.compile()
    return nc


def _route(x, Wg, bg):
    """Host gating: returns flat tokens, per-expert index lists, combine wts."""
    xf = np.asarray(x, dtype=np.float32).reshape(T, H)
    logits = xf @ np.asarray(Wg, dtype=np.float32) + np.asarray(bg, dtype=np.float32)
    m = logits.max(-1, keepdims=True)
    p = np.exp(logits - m)
    p /= p.sum(-1, keepdims=True)
    order = np.argsort(-p, axis=-1)
    topi = order[:, :2]
    mask = np.zeros_like(p, dtype=bool)
    np.put_along_axis(mask, topi, True, axis=-1)
    comb = (p * mask).astype(np.float32)  # [T, E] raw softmax prob, top-2 gated
    idx_lists = [np.nonzero(mask[:, e])[0] for e in range(E)]
    return xf, idx_lists, comb


def _pack(idx_lists, n_a):
    """Assign (expert, token) pairs to per-core A (n_a*AW) / B (BW) regions.

    Greedy: expert c's first A_cap tokens on core c; overflow pieces split
    into <=BW chunks, first-fit into the 8 B slots.
    """
    a_cap = n_a * AW
    cores = []
    pieces = []
    for e in range(E):
        ix = idx_lists[e]
        cores.append({"pe": e, "pix": ix[:a_cap], "be": e, "bix": ix[:0]})
        if len(ix) > a_cap:
            pieces.append((e, ix[a_cap:]))
    pieces.sort(key=lambda p: -len(p[1]))
    free = list(range(E))
    for e, rem in pieces:
        while len(rem) > 0:
            assert free, "overflow does not fit; raise n_a"
            c = free.pop(0)
            cores[c]["be"] = e
            cores[c]["bix"] = rem[:BW]
            rem = rem[BW:]
    return cores


def kernel(x, Wg, bg, W1, b1, W2, b2):
    global LAST_RESULT
    xf, idx_lists, comb = _route(x, Wg, bg)
    total_tiles = sum(-(-len(ix) // 128) for ix in idx_lists)
    n_a = max(-(-(total_tiles - E * (BW // 128)) // (E * (AW // 128))), 2)

    def b_slots_needed(n):
        return sum(-(-max(0, len(ix) - n * AW) // BW) for ix in idx_lists)

    while b_slots_needed(n_a) > E:
        n_a += 1
    C = n_a * AW + BW

    b1 = np.asarray(b1, dtype=np.float32)
    b2 = np.asarray(b2, dtype=np.float32)
    b1_zero = not np.any(b1)
    b2_zero = not np.any(b2)
    key = ("nc", n_a, b1_zero, b2_zero)
    if key not in _CACHE:
        _CACHE[key] = _build(n_a, b1_zero, b2_zero)
    nc = _CACHE[key]

    W1 = np.asarray(W1, dtype=np.float32)
    W2 = np.asarray(W2, dtype=np.float32)
    w1p = {}
    w2p = {}
    for e in range(E):
        w1p[e] = np.ascontiguousarray(
            np.transpose(W1[e].reshape(4, 128, F), (1, 0, 2)).astype(ml_dtypes.bfloat16)
        )
        w2p[e] = np.ascontiguousarray(
            np.transpose(W2[e].reshape(F // 128, 128, H), (1, 0, 2)).astype(
                ml_dtypes.bfloat16
            )
        )

    cores = _pack(idx_lists, n_a)
    a_cap = n_a * AW
    in_maps = []
    for cdesc in cores:
        pe, be = cdesc["pe"], cdesc["be"]
        pix, bix = cdesc["pix"], cdesc["bix"]
        pad = np.zeros(C, dtype=np.int64)
        pad[: len(pix)] = pix
        pad[a_cap : a_cap + len(bix)] = bix
        xe = xf[pad]  # [C, H] fp32 (pad rows = token 0, comb 0)
        xgc = np.ascontiguousarray(
            np.transpose(xe.T.reshape(4, 128, C), (1, 0, 2)).astype(ml_dtypes.bfloat16)
        )
        cw = np.zeros(C, dtype=np.float32)
        cw[: len(pix)] = comb[pix, pe]
        cw[a_cap : a_cap + len(bix)] = comb[bix, be]
        in_maps.append(
            {
                "xg": xgc,
                "w1a": w1p[pe],
                "w2a": w2p[pe],
                "w1b": w1p[be],
                "w2b": w2p[be],
                "b1t": np.ascontiguousarray(
                    np.stack([b1[pe], b1[be]]).reshape(2, F // 128, 128).swapaxes(1, 2)
                ),
                "b2r": np.ascontiguousarray(
                    np.broadcast_to(
                        np.stack([b2[pe], b2[be]])[:, None, :], (2, 128, H)
                    ).copy()
                ),
                # comb[p, j] pairs with output row j*128+p of yout
                "comb": np.ascontiguousarray(cw.reshape(C // 128, 128).T),
            }
        )

    import os

    trace = bool(os.environ.get("BASS_TRACE"))
    res = bass_utils.run_bass_kernel_spmd(
        nc, in_maps, core_ids=list(range(E)), trace=trace
    )
    LAST_RESULT = res
    out = np.zeros((T, H), dtype=np.float32)
    for c, cdesc in enumerate(cores):
        y = np.asarray(res.results[c]["yout"], dtype=np.float32)
        pix, bix = cdesc["pix"], cdesc["bix"]
        out[pix] += y[: len(pix)]
        if len(bix):
            out[bix] += y[a_cap : a_cap + len(bix)]
    return out.reshape(8, 2048, H)
